# revision 5
# baseline (speedup 1.0000x reference)
"""Boundary loss kernel for Trainium2 (8 NeuronCores, SPMD).

loss = mean(sigmoid(pred) * EDT(target)) for pred/target [4,1,512,512].

Strategy:
  - The exact EDT dist2[y,x] = min over foreground (dy,dx) of dy^2+dx^2 is
    computed with a windowed separable min (window radius K): phase A does the
    vertical windowed min on a transposed layout (shifts along the free dim),
    a TensorE transpose flips the layout, phase B does the horizontal windowed
    min. If every resulting dist2 <= K^2 the windowed result provably equals
    the exact EDT (any pixel with true distance <= K has its nearest
    foreground inside the window). The kernel reduces max(dist2-K^2, 0) so the
    host can verify that certificate; on failure (can't happen for ~50%-dense
    random masks, where max dist is ~3) we fall back to an exact numpy EDT.
  - Sharding: core c handles sample c//2, row-half c%2 (256 rows + K halo).
  - Host pre-packs the mask as nb^T = BIG*(1-mask) bf16 [512, 262] so the
    device consumes it directly; pred ships as f32 [256, 512].
  - Device output per core: [128, 2] f32 = per-partition sums of
    sigmoid(pred)*dist and of the certificate residual; host finishes the
    tiny 256-element reduction and normalization.
"""

import sys

sys.path.insert(0, "/opt/trn_rl_repo")

import numpy as np
import ml_dtypes

K = 3
BIG = 16384.0
B, H, W = 4, 512, 512
HALF = 256
HALO = HALF + 2 * K  # 262

_compiled = None


def _build_bass():
    import concourse.bacc as bacc
    import concourse.tile as tile
    from concourse import mybir
    from concourse.masks import make_identity

    # Bacc (not plain Bass): its compile pipeline runs
    # generate_event_semaphores, which splits multi-wait drains that TRN2
    # codegen otherwise rejects ("Too many sync wait commands").
    nc = bacc.Bacc(None)
    dt = mybir.dt
    Alu = mybir.AluOpType
    Act = mybir.ActivationFunctionType

    nbt_d = nc.dram_tensor("nbt", [W, HALO], dt.bfloat16, kind="ExternalInput")
    pred_d = nc.dram_tensor("pred", [HALF, W], dt.float32, kind="ExternalInput")
    out_d = nc.dram_tensor("out", [128, 2], dt.float32, kind="ExternalOutput")

    with tile.TileContext(nc) as tc:
        with (
            tc.tile_pool(name="sb", bufs=1) as sb,
            tc.tile_pool(name="ps", bufs=2, space="PSUM") as ps,
        ):
            ident = sb.tile([128, 128], dt.bfloat16)
            make_identity(nc, ident[:])

            nbt = sb.tile([128, 4, HALO], dt.bfloat16)
            nc.sync.dma_start(
                out=nbt[:], in_=nbt_d[:].rearrange("(t p) h -> p t h", p=128)
            )
            pred_sb = sb.tile([128, 2, W], dt.float32)
            nc.sync.dma_start(
                out=pred_sb[:], in_=pred_d[:].rearrange("(j p) w -> p j w", p=128)
            )

            # Phase A: vertical windowed min in transposed layout.
            # acc_v[p, t, h] = min_{|dy|<=K} nbt[p, t, h+K+dy] + dy^2
            #               = (vertical squared distance, or >= BIG)
            acc_v = sb.tile([128, 4, HALF], dt.bfloat16)
            nc.vector.tensor_copy(acc_v[:], nbt[:, :, K : K + HALF])
            for d in range(1, K + 1):
                for sh in (d, -d):
                    nc.vector.scalar_tensor_tensor(
                        out=acc_v[:],
                        in0=nbt[:, :, K + sh : K + sh + HALF],
                        scalar=float(d * d),
                        in1=acc_v[:],
                        op0=Alu.add,
                        op1=Alu.min,
                    )

            # Transpose [w, h] -> [h, w] via TensorE, 128x128 blocks.
            m2v = sb.tile([128, 2, W], dt.bfloat16)
            for j in range(2):
                pt = ps.tile([128, 512], dt.bfloat16)
                for t in range(4):
                    nc.tensor.transpose(
                        out=pt[:, t * 128 : (t + 1) * 128],
                        in_=acc_v[:, t, j * 128 : (j + 1) * 128],
                        identity=ident[:],
                    )
                nc.scalar.copy(out=m2v[:, j, :], in_=pt[:])

            # Phase B: horizontal windowed min in natural layout.
            acc_h = sb.tile([128, 2, W], dt.bfloat16)
            nc.vector.tensor_copy(acc_h[:], m2v[:])
            for d in range(1, K + 1):
                for dx in (d, -d):
                    lo = max(0, -dx)
                    hi = W - max(0, dx)
                    nc.vector.scalar_tensor_tensor(
                        out=acc_h[:, :, lo:hi],
                        in0=m2v[:, :, lo + dx : hi + dx],
                        scalar=float(d * d),
                        in1=acc_h[:, :, lo:hi],
                        op0=Alu.add,
                        op1=Alu.min,
                    )

            out_sb = sb.tile([128, 2], dt.float32)

            # Certificate: sum over pixels of max(dist2 - K^2, 0); zero iff
            # every dist2 <= K^2, i.e. the windowed EDT is exact.
            cert_junk = sb.tile([128, 2, W], dt.bfloat16)
            nc.vector.tensor_scalar(
                out=cert_junk[:],
                in0=acc_h[:],
                scalar1=-float(K * K),
                scalar2=0.0,
                op0=Alu.add,
                op1=Alu.max,
                accum_out=out_sb[:, 1:2],
            )

            dist = sb.tile([128, 2, W], dt.float32)
            nc.scalar.activation(out=dist[:], in_=acc_h[:], func=Act.Sqrt)
            sig = sb.tile([128, 2, W], dt.float32)
            nc.scalar.activation(out=sig[:], in_=pred_sb[:], func=Act.Sigmoid)

            prod_junk = sb.tile([128, 2, W], dt.float32)
            nc.vector.scalar_tensor_tensor(
                out=prod_junk[:],
                in0=sig[:],
                scalar=1.0,
                in1=dist[:],
                op0=Alu.mult,
                op1=Alu.mult,
                accum_out=out_sb[:, 0:1],
            )

            nc.sync.dma_start(out=out_d[:], in_=out_sb[:])

    # Run Bacc's compile pipeline (register allocation, event-semaphore
    # splitting); the axon PJRT path serializes nc.m as-is.
    nc.finalize()
    return nc


def _exact_loss_numpy(pred, target):
    """Exact fallback, matching reference.py semantics bit-for-bit in spirit."""
    mask = target[:, 0].astype(np.float32)
    b, h, w = mask.shape
    big = np.float32(h + w)
    rows = np.arange(h, dtype=np.float32)[None, :, None]
    fg = mask > 0
    last = np.maximum.accumulate(np.where(fg, rows, -big), axis=1)
    nxt = np.minimum.accumulate(np.where(fg, rows, 3 * big)[:, ::-1], axis=1)[:, ::-1]
    g = np.minimum(np.minimum(rows - last, nxt - rows), big)
    g2 = (g * g).astype(np.float32)
    cols = np.arange(w, dtype=np.float32)
    diff2 = (cols[:, None] - cols[None, :]) ** 2
    dist = np.empty((b, h, w), np.float32)
    for bi in range(b):
        for r0 in range(0, h, 64):
            blk = g2[bi, r0 : r0 + 64]
            dist[bi, r0 : r0 + 64] = np.sqrt(
                (diff2[None, :, :] + blk[:, None, :]).min(-1)
            )
    has_fg = fg.any(axis=(1, 2))
    dist = np.where(has_fg[:, None, None], dist, 0.0)
    p = 1.0 / (1.0 + np.exp(-pred[:, 0].astype(np.float64)))
    return np.float32((p * dist).mean())


def _prep_in_maps(pred, target):
    bf16 = ml_dtypes.bfloat16
    mask = (target[:, 0] > 0).astype(np.float32)  # [B, H, W]
    in_maps = []
    for c in range(8):
        s, j = c // 2, c % 2
        r0 = j * HALF
        halo = np.zeros((HALO, W), np.float32)
        lo, hi = r0 - K, r0 + HALF + K
        slo, shi = max(lo, 0), min(hi, H)
        halo[slo - lo : shi - lo] = mask[s, slo:shi]
        nbt = np.ascontiguousarray((BIG * (1.0 - halo)).T).astype(bf16)
        predh = np.ascontiguousarray(pred[s, 0, r0 : r0 + HALF, :]).astype(np.float32)
        in_maps.append({"nbt": nbt, "pred": predh})
    return in_maps


def kernel_with_results(pred, target, trace=False):
    """Returns (loss, BassKernelResults)."""
    global _compiled
    from concourse.bass_utils import run_bass_kernel_spmd

    if _compiled is None:
        _compiled = _build_bass()
    nc = _compiled

    in_maps = _prep_in_maps(pred, target)
    bkr = run_bass_kernel_spmd(nc, in_maps, core_ids=list(range(8)), trace=trace)

    has_fg = (target[:, 0] > 0).any(axis=(1, 2))  # [B]
    total = np.float64(0.0)
    cert = 0.0
    for c in range(8):
        s = c // 2
        if not has_fg[s]:
            continue
        out = bkr.results[c]["out"]  # [128, 2] f32
        total += np.float64(out[:, 0].sum(dtype=np.float64))
        cert = max(cert, float(out[:, 1].sum(dtype=np.float64)))

    if cert > 1e-3:
        # Windowed EDT not certified exact for this input; fall back.
        return _exact_loss_numpy(pred, target), bkr

    loss = np.array(total / (B * 1 * H * W), dtype=np.float32)
    return loss, bkr


def kernel(pred, target):
    loss, _ = kernel_with_results(pred, target)
    return loss


# revision 10
# speedup vs baseline: 1.0483x; 1.0483x over previous
"""Boundary loss kernel for Trainium2 (8 NeuronCores, SPMD).

loss = mean(sigmoid(pred) * EDT(target)) for pred/target [4,1,512,512].

Algorithm:
  The exact EDT dist2[y,x] = min over foreground (dy,dx) of dy^2+dx^2 is
  computed with a windowed separable min (window radius K=3): phase A does the
  vertical windowed min on a transposed [w, h] layout (shifts along the free
  dim), a TensorE transpose flips to [h, w], phase B does the horizontal
  windowed min. If every resulting dist2 <= K^2, the windowed result provably
  equals the exact EDT (a pixel with true distance <= K has its nearest
  foreground inside the window). The kernel also reduces
  sum(max(dist2 - K^2, 0)) as that exactness certificate; if it is nonzero
  (impossible for ~50%-dense random masks, where max distance is ~3) the host
  falls back to an exact numpy EDT — still correct, just slower on the host.

Sharding: core c handles sample c//2, row-half c%2 (256 rows + halo).

Performance notes:
  - DVE bf16 tensor ops hit 2x mode only with 4-byte-aligned access patterns,
    so all shifts are arranged at even element offsets: data sits at base
    offset PAD=4 and odd shifts read a one-element-shifted copy (nbtR/m2vRp)
    built by the otherwise-idle GpSimd/ScalarE engines.
  - Host pre-packs inputs in the exact SBUF tile layout so DMAs are fully
    contiguous per partition.
  - Certificate reduction runs on GpSimd, sqrt/sigmoid on ScalarE, min-chains
    and the final fused multiply+sum on VectorE.
"""

import sys

sys.path.insert(0, "/opt/trn_rl_repo")

import numpy as np
import ml_dtypes

K = 3
BIG = 16384.0
PAD = 4
B, H, W = 4, 512, 512
HALF = 256
HALO = HALF + 2 * PAD  # 264

_compiled = None


def _build_bass():
    import concourse.bacc as bacc
    import concourse.tile as tile
    from concourse import mybir
    from concourse.masks import make_identity

    # Bacc (not plain Bass): its compile pipeline runs register allocation
    # and generate_event_semaphores (splits multi-wait drains TRN2 codegen
    # rejects with "Too many sync wait commands").
    nc = bacc.Bacc(None)
    dt = mybir.dt
    Alu = mybir.AluOpType
    Act = mybir.ActivationFunctionType

    # Inputs are host-packed in SBUF layout: nbt[p, t, h] = BIG*(1-mask) at
    # column w = t*128+p, halo row h; pred[p, j, w] = logits at row j*128+p.
    nbt_d = nc.dram_tensor("nbt", [128, 4 * HALO], dt.bfloat16, kind="ExternalInput")
    pred_d = nc.dram_tensor("pred", [128, 2 * W], dt.float32, kind="ExternalInput")
    out_d = nc.dram_tensor("out", [128, 4], dt.float32, kind="ExternalOutput")

    with tile.TileContext(nc) as tc:
        with (
            tc.tile_pool(name="sb", bufs=1) as sb,
            tc.tile_pool(name="ps", bufs=2, space="PSUM") as ps,
        ):
            nbt = sb.tile([128, 4, HALO], dt.bfloat16)
            nc.sync.dma_start(out=nbt[:], in_=nbt_d[:].rearrange("p (t h) -> p t h", t=4))
            pred_sb = sb.tile([128, 2, W], dt.float32)
            nc.sync.dma_start(out=pred_sb[:], in_=pred_d[:].rearrange("p (j w) -> p j w", j=2))

            ident = sb.tile([128, 128], dt.bfloat16)
            make_identity(nc, ident[:])

            # Shifted copy for odd-dy reads: nbtR[h] = nbt[h+1]. On ScalarE,
            # first in its queue so it's ready when phase A op3 needs it.
            nbtR = sb.tile([128, 4, HALO], dt.bfloat16)
            nc.scalar.copy(nbtR[:, :, 0 : HALO - 1], nbt[:, :, 1:HALO])

            # Sigmoid only needs pred: issue early so ScalarE does it while
            # VectorE runs phase A.
            sig = sb.tile([128, 2, W], dt.float32)
            nc.scalar.activation(out=sig[:], in_=pred_sb[:], func=Act.Sigmoid)

            # Phase A: vertical windowed min. Image row r0+h' is nbt index
            # PAD+h'; acc_v = min_dy nbt[PAD+h'+dy] + dy^2. Every in0 slice
            # starts at an even element offset (4B-aligned, DVE 2x mode).
            acc_v = sb.tile([128, 4, HALF], dt.bfloat16)
            P = PAD
            stt = nc.vector.scalar_tensor_tensor
            # dy=+2 fused with dy=0 (first op, no init needed)
            stt(out=acc_v[:], in0=nbt[:, :, P + 2 : P + 2 + HALF], scalar=4.0,
                in1=nbt[:, :, P : P + HALF], op0=Alu.add, op1=Alu.min)
            for in_t, off, d2 in (
                (nbt, P - 2, 4.0),   # dy=-2
                (nbtR, P, 1.0),      # dy=+1: nbt[h+1] = nbtR[h]
                (nbtR, P - 2, 1.0),  # dy=-1: nbt[h-1] = nbtR[h-2]
                (nbtR, P + 2, 9.0),  # dy=+3: nbt[h+3] = nbtR[h+2]
                (nbtR, P - 4, 9.0),  # dy=-3: nbt[h-3] = nbtR[h-4]
            ):
                stt(out=acc_v[:], in0=in_t[:, :, off : off + HALF], scalar=d2,
                    in1=acc_v[:], op0=Alu.add, op1=Alu.min)

            # Transpose [w, h] -> [h, w] via TensorE; land in padded m2vp
            # (data at [4, 516), pads = BIG) plus the one-element-shifted
            # twin m2vRp[w] = m2vp[w+1] (data at [3, 515)).
            m2vp = sb.tile([128, 2, 520], dt.bfloat16)
            m2vRp = sb.tile([128, 2, 520], dt.bfloat16)
            nc.gpsimd.memset(m2vp[:], BIG)
            nc.gpsimd.memset(m2vRp[:], BIG)
            for j in range(2):
                pt = ps.tile([128, 512], dt.bfloat16)
                for t in range(4):
                    nc.tensor.transpose(
                        out=pt[:, t * 128 : (t + 1) * 128],
                        in_=acc_v[:, t, j * 128 : (j + 1) * 128],
                        identity=ident[:],
                    )
                nc.scalar.copy(out=m2vp[:, j, 4:516], in_=pt[:])
                nc.scalar.copy(out=m2vRp[:, j, 3:515], in_=pt[:])

            # Phase B: horizontal windowed min, full-width ops, all offsets
            # even (m2vp data base 4; odd dx via m2vRp at base 3).
            acc_h = sb.tile([128, 2, W], dt.bfloat16)
            stt(out=acc_h[:], in0=m2vp[:, :, 6:518], scalar=4.0,
                in1=m2vp[:, :, 4:516], op0=Alu.add, op1=Alu.min)  # dx=+2, 0
            for in_t, off, d2 in (
                (m2vp, 2, 4.0),   # dx=-2
                (m2vRp, 4, 1.0),  # dx=+1
                (m2vRp, 2, 1.0),  # dx=-1
                (m2vRp, 6, 9.0),  # dx=+3
                (m2vRp, 0, 9.0),  # dx=-3
            ):
                stt(out=acc_h[:], in0=in_t[:, :, off : off + W], scalar=d2,
                    in1=acc_h[:], op0=Alu.add, op1=Alu.min)

            out_sb = sb.tile([128, 4], dt.float32)
            nc.gpsimd.memset(out_sb[:], 0.0)

            # Exactness certificate (walrus rejects tensor_scalar on GpSimd).
            cert_junk = sb.tile([128, 2, W], dt.bfloat16)
            nc.vector.tensor_scalar(
                out=cert_junk[:], in0=acc_h[:],
                scalar1=-float(K * K), scalar2=0.0,
                op0=Alu.add, op1=Alu.max,
                accum_out=out_sb[:, 2:3],
            )

            # Tail, split per row-half so stt(j0) overlaps sqrt(j1).
            dist = sb.tile([128, 2, W], dt.float32)
            prod_junk = sb.tile([128, 2, W], dt.float32)
            for j in range(2):
                nc.scalar.activation(out=dist[:, j, :], in_=acc_h[:, j, :], func=Act.Sqrt)
                nc.vector.scalar_tensor_tensor(
                    out=prod_junk[:, j, :], in0=sig[:, j, :], scalar=1.0,
                    in1=dist[:, j, :], op0=Alu.mult, op1=Alu.mult,
                    accum_out=out_sb[:, j : j + 1],
                )

            nc.sync.dma_start(out=out_d[:], in_=out_sb[:])

    nc.finalize()
    return nc


def _exact_loss_numpy(pred, target):
    """Exact fallback, matching reference.py semantics."""
    mask = target[:, 0].astype(np.float32)
    b, h, w = mask.shape
    big = np.float32(h + w)
    rows = np.arange(h, dtype=np.float32)[None, :, None]
    fg = mask > 0
    last = np.maximum.accumulate(np.where(fg, rows, -big), axis=1)
    nxt = np.minimum.accumulate(np.where(fg, rows, 3 * big)[:, ::-1], axis=1)[:, ::-1]
    g = np.minimum(np.minimum(rows - last, nxt - rows), big)
    g2 = (g * g).astype(np.float32)
    cols = np.arange(w, dtype=np.float32)
    diff2 = (cols[:, None] - cols[None, :]) ** 2
    dist = np.empty((b, h, w), np.float32)
    for bi in range(b):
        for r0 in range(0, h, 64):
            blk = g2[bi, r0 : r0 + 64]
            dist[bi, r0 : r0 + 64] = np.sqrt(
                (diff2[None, :, :] + blk[:, None, :]).min(-1)
            )
    has_fg = fg.any(axis=(1, 2))
    dist = np.where(has_fg[:, None, None], dist, 0.0)
    p = 1.0 / (1.0 + np.exp(-pred[:, 0].astype(np.float64)))
    return np.float32((p * dist).mean())


def _prep_in_maps(pred, target):
    bf16 = ml_dtypes.bfloat16
    mask = (target[:, 0] > 0).astype(np.float32)  # [B, H, W]
    in_maps = []
    for c in range(8):
        s, j = c // 2, c % 2
        r0 = j * HALF
        halo = np.zeros((HALO, W), np.float32)
        lo, hi = r0 - PAD, r0 + HALF + PAD
        slo, shi = max(lo, 0), min(hi, H)
        halo[slo - lo : shi - lo] = mask[s, slo:shi]
        # nbt[p, t, h] for column w = t*128+p -> pack as [128, 4*HALO]
        nbt_wh = (BIG * (1.0 - halo)).T  # [W, HALO]
        nbt = np.ascontiguousarray(
            nbt_wh.reshape(4, 128, HALO).transpose(1, 0, 2).reshape(128, 4 * HALO)
        ).astype(bf16)
        # pred[p, j2, w] for row r0 + j2*128 + p -> pack as [128, 2*W]
        ph = pred[s, 0, r0 : r0 + HALF, :].astype(np.float32)
        predh = np.ascontiguousarray(
            ph.reshape(2, 128, W).transpose(1, 0, 2).reshape(128, 2 * W)
        )
        in_maps.append({"nbt": nbt, "pred": predh})
    return in_maps


def kernel_with_results(pred, target, trace=False):
    """Returns (loss, BassKernelResults)."""
    global _compiled
    from concourse.bass_utils import run_bass_kernel_spmd

    if _compiled is None:
        _compiled = _build_bass()
    nc = _compiled

    in_maps = _prep_in_maps(pred, target)
    bkr = run_bass_kernel_spmd(nc, in_maps, core_ids=list(range(8)), trace=trace)

    has_fg = (target[:, 0] > 0).any(axis=(1, 2))  # [B]
    total = np.float64(0.0)
    cert = 0.0
    for c in range(8):
        s = c // 2
        if not has_fg[s]:
            continue
        out = bkr.results[c]["out"]  # [128, 4] f32
        total += np.float64(out[:, 0:2].sum(dtype=np.float64))
        cert = max(cert, float(out[:, 2].sum(dtype=np.float64)))

    if cert > 1e-3:
        # Windowed EDT not certified exact for this input; fall back.
        return _exact_loss_numpy(pred, target), bkr

    loss = np.array(total / (B * 1 * H * W), dtype=np.float32)
    return loss, bkr


def kernel(pred, target):
    loss, _ = kernel_with_results(pred, target)
    return loss
